# revision 10
# baseline (speedup 1.0000x reference)
"""Trainium2 Bass kernel for the nn_MultiHeadAttention problem.

Sharding: 8 cores = 4 batches x 2 head-groups (8 heads each).
Each core computes, for its (batch b, head-group g):
  qT/kT = (X @ W)^T via PE-transposed X, v = X @ W_V (natural),
  per head: S^T[k,q] = k_h^T.T-contract, E = exp(S^T/8) with fused
  column-sum on the ScalarE accumulator (softmax over the QUERY axis is a
  free-axis reduction in the S^T layout), V' = v / colsum, O^T = V'^T-AV,
  yT_partial = W_O_loc^T @ O^T.
Host sums the two head-group partials per batch and transposes.

All matmuls run as float32r (1-pass FP22) for full PE rate.
"""

import numpy as np

P = 128
N = 2048          # tokens
DM = 1024         # d_model
NL = 512          # local projection width (8 heads x 64)
D = 64            # head dim
NT = N // P       # 16 token chunks
DMC = DM // P     # 8 d_model chunks
NJ = NL // P      # 4 head pairs
QS = 512          # matmul free-dim slice
NQS = N // QS     # 4

_CACHE = {}


def _apply_tile_drain_patch():
    """This container's walrus codegen accepts at most ONE sync-wait per
    instruction ("Too many sync wait commands"). Tile's tail drain collects
    the whole global clock on a single SP Drain; pre-split it across SP NOPs
    here so scheduling sees it, and _split_sync_waits() handles the rest of
    the program post-scheduling."""
    import bass_rust as _br
    from concourse.tile import TileContext

    if getattr(TileContext, "_drain_patch_applied", False):
        return

    def _patched(self, tick_clock, wait_clock):
        nc = self.nc
        nops = [nc.sync.nop() for _ in range(40)]
        drain_inst = nc.sync.drain()
        wait_clock.add_sem_waits(
            drain_inst.ins, _br.ScopedClock({None: tick_clock.global_clock})
        )
        si = drain_inst.ins.sync_info
        if si is not None and len(si.on_wait) > 1:
            waits = list(si.on_wait)
            assert len(waits) <= 41, f"too many tail waits: {len(waits)}"
            si.on_wait = waits[:1]
            for i, w in enumerate(waits[1:]):
                nops[i].ins.sync_info = _br.SyncInfo(on_wait=[w], on_update=[])
        nc.all_engine_barrier()
        assert self.sems is not None
        popped = nc._tile_sem_poison_stack.pop()
        assert popped is self._sem_poison
        nc.clear_and_free_semaphores(list(self.sems.allocated().values()))
        nc.all_engine_barrier()

    TileContext._drain_and_barrier = _patched
    TileContext._drain_patch_applied = True


def _split_sync_waits(nc):
    """Walrus in this image allows max one sync-wait per instruction. Hoist
    extra waits onto same-engine NOPs inserted immediately before the
    instruction (engine streams are serial, so semantics are preserved)."""
    import bass_rust as _br
    import concourse.mybir as mybir

    for fn in nc.m.functions:
        for blk in fn.blocks:
            new_insts = []
            changed = False
            for inst in blk.instructions:
                si = inst.sync_info
                if si is not None and len(si.on_wait) > 1:
                    waits = list(si.on_wait)
                    for k, w in enumerate(waits[:-1]):
                        nop = mybir.InstNoOp(
                            name=f"{inst.name}-waitsplit-{k}", ins=[], outs=[])
                        nop.engine = inst.engine
                        nop.sync_info = _br.SyncInfo(on_wait=[w], on_update=[])
                        new_insts.append(nop)
                    si.on_wait = waits[-1:]
                    changed = True
                new_insts.append(inst)
            if changed:
                blk.instructions = new_insts


def _build_nc():
    import concourse.bass as bass
    import concourse.mybir as mybir
    from concourse.tile import TileContext

    _apply_tile_drain_patch()

    f32 = mybir.dt.float32
    f32r = mybir.dt.float32r
    bf16 = mybir.dt.bfloat16
    Exp = mybir.ActivationFunctionType.Exp

    nc = bass.Bass("TRN2", target_bir_lowering=False, debug=False, num_devices=8)

    xq = nc.dram_tensor("xq", [N, DM], f32r, kind="ExternalInput")
    xk = nc.dram_tensor("xk", [N, DM], f32r, kind="ExternalInput")
    xv = nc.dram_tensor("xv", [N, DM], f32r, kind="ExternalInput")
    wq = nc.dram_tensor("wq", [DM, NL], f32r, kind="ExternalInput")
    wk = nc.dram_tensor("wk", [DM, NL], f32r, kind="ExternalInput")
    wv = nc.dram_tensor("wv", [DM, NL], f32r, kind="ExternalInput")
    wo = nc.dram_tensor("wo", [NL, DM], f32r, kind="ExternalInput")
    ident_d = nc.dram_tensor("ident", [P, P], f32r, kind="ExternalInput")
    yT = nc.dram_tensor("yT", [DM, N], f32, kind="ExternalOutput")

    with TileContext(nc) as tc:
        with (
            tc.tile_pool(name="const", bufs=1) as constp,
            tc.tile_pool(name="perm", bufs=1) as perm,
        ):
            ident = constp.tile([P, P], f32r)
            nc.sync.dma_start(out=ident[:], in_=ident_d[:])

            # persistent products
            qT = perm.tile([P, NJ, N], f32r)    # qT[p, j, t] = q[t, j*128+p]
            kT = perm.tile([P, NJ, N], f32r)
            v = perm.tile([P, NT, NL], f32r)    # v[p, tc, n] = v[tc*128+p, n]

            # ---------------- setup: transposes + projections ----------------
            with (
                tc.tile_pool(name="xTp", bufs=1) as xTp,
                tc.tile_pool(name="stagep", bufs=3) as stagep,
                tc.tile_pool(name="wp", bufs=1) as wp,
                tc.tile_pool(name="pspt", bufs=1, space="PSUM") as pspt,
                tc.tile_pool(name="pspj", bufs=1, space="PSUM") as pspj,
            ):
                for phase, (x_d, w_d) in enumerate([(xq, wq), (xk, wk), (xv, wv)]):
                    # X^T: [p, c, t] = X[t, c*128+p]
                    xT = xTp.tile([P, DMC, N], f32r, tag="xT", name="xT")
                    for csl in range(2):
                        for tsl in range(NQS):
                            # 4 psum tiles, one per c in this column-slice group;
                            # each tag allocated once per (csl, tsl) and copied
                            # out before the next group reallocates it.
                            pst = {}
                            for ci in range(4):
                                pst[ci] = pspt.tile([P, QS], f32r,
                                                    tag=f"tp{ci}", name="pst")
                            for tci in range(4):
                                tcg = tsl * 4 + tci
                                xst = stagep.tile([P, QS], f32r, tag="xst",
                                                  name="xst")
                                nc.sync.dma_start(
                                    out=xst[:],
                                    in_=x_d[tcg * P:(tcg + 1) * P,
                                            csl * QS:(csl + 1) * QS])
                                for ci in range(4):
                                    nc.tensor.transpose(
                                        pst[ci][:, tci * P:(tci + 1) * P],
                                        xst[:, ci * P:(ci + 1) * P],
                                        ident[:],
                                    )
                            for ci in range(4):
                                c = csl * 4 + ci
                                nc.vector.tensor_copy(
                                    out=xT[:, c, tsl * QS:(tsl + 1) * QS],
                                    in_=pst[ci][:])

                    w_sb = wp.tile([P, DMC, NL], f32r, tag="w", name="w_sb")
                    nc.sync.dma_start(
                        out=w_sb[:], in_=w_d.rearrange("(c p) n -> p c n", p=P))

                    if phase < 2:
                        # q^T / k^T: out[j*128+p, t], lhsT = W chunk, rhs = X^T
                        dst = qT if phase == 0 else kT
                        for j in range(NJ):
                            psj = [pspj.tile([P, QS], f32, tag=f"pj{t}",
                                             name="psj") for t in range(NQS)]
                            for c in range(DMC):
                                for tsl in range(NQS):
                                    nc.tensor.matmul(
                                        psj[tsl][:],
                                        w_sb[:, c, j * P:(j + 1) * P],
                                        xT[:, c, tsl * QS:(tsl + 1) * QS],
                                        start=(c == 0), stop=(c == DMC - 1))
                            for tsl in range(NQS):
                                nc.vector.tensor_copy(
                                    out=dst[:, j, tsl * QS:(tsl + 1) * QS],
                                    in_=psj[tsl][:])
                    else:
                        # v natural: out[tc*128+p, n], lhsT = X^T chunk, rhs = W
                        for tcg in range(NT):
                            psv = pspj.tile([P, NL], f32, tag=f"pj{tcg % 4}",
                                            name="psv")
                            for c in range(DMC):
                                nc.tensor.matmul(
                                    psv[:],
                                    xT[:, c, tcg * P:(tcg + 1) * P],
                                    w_sb[:, c, :],
                                    start=(c == 0), stop=(c == DMC - 1))
                            nc.vector.tensor_copy(out=v[:, tcg, :], in_=psv[:])

            # ---------------- attention core ----------------
            with tc.tile_pool(name="attnp", bufs=1) as attnp:
                oT = attnp.tile([P, NJ, N], f32r)   # oT[p, j, t]; rows = pair d
                with (
                    tc.tile_pool(name="ep", bufs=2) as ep,
                    tc.tile_pool(name="smallp", bufs=3) as smallp,
                    tc.tile_pool(name="pss", bufs=1, space="PSUM") as pss,
                    tc.tile_pool(name="psav", bufs=1, space="PSUM") as psav,
                ):
                    for j in range(NJ):
                        av = [psav.tile([P, 2 * QS], f32, tag=f"av{qh}",
                                        name="av") for qh in range(2)]
                        for kc in range(NT):
                            e_t = []
                            vp = smallp.tile([P, P], bf16, tag="vp", name="vp")
                            for hh in range(2):
                                rows = slice(hh * D, (hh + 1) * D)
                                acc = smallp.tile([P, 2], f32, tag=f"acc{hh}",
                                                  name="acc")
                                e = ep.tile([P, N], bf16, tag=f"e{hh}", name="e")
                                for qh in range(2):
                                    s = pss.tile([P, 2 * QS], f32, tag=f"s{hh}",
                                                 name="s")
                                    for qs in range(2):
                                        nc.tensor.matmul(
                                            s[:, qs * QS:(qs + 1) * QS],
                                            kT[rows, j, kc * P:(kc + 1) * P],
                                            qT[rows, j,
                                                 qh * 1024 + qs * QS:
                                                 qh * 1024 + (qs + 1) * QS],
                                            start=True, stop=True)
                                    nc.scalar.activation(
                                        out=e[:, qh * 1024:(qh + 1) * 1024],
                                        in_=s[:],
                                        func=Exp,
                                        scale=0.125,
                                        accum_out=acc[:, qh:qh + 1])
                                cs = smallp.tile([P, 1], f32, tag=f"cs{hh}",
                                                 name="cs")
                                nc.vector.tensor_add(
                                    out=cs[:], in0=acc[:, 0:1], in1=acc[:, 1:2])
                                rec = smallp.tile([P, 1], f32, tag=f"rec{hh}",
                                                  name="rec")
                                nc.vector.reciprocal(rec[:], cs[:])
                                nc.vector.tensor_scalar_mul(
                                    vp[:, hh * D:(hh + 1) * D],
                                    v[:, kc,
                                      j * P + hh * D: j * P + (hh + 1) * D],
                                    rec[:])
                                e_t.append(e)
                            for hh in range(2):
                                cols = slice(hh * D, (hh + 1) * D)
                                for qh in range(2):
                                    for qs in range(2):
                                        nc.tensor.matmul(
                                            av[qh][cols, qs * QS:(qs + 1) * QS],
                                            vp[:, cols],
                                            e_t[hh][:,
                                                      qh * 1024 + qs * QS:
                                                      qh * 1024 + (qs + 1) * QS],
                                            start=(kc == 0),
                                            stop=(kc == NT - 1))
                        for qh in range(2):
                            nc.vector.tensor_copy(
                                out=oT[:, j, qh * 1024:(qh + 1) * 1024],
                                in_=av[qh][:])

                # ---------------- output projection ----------------
                with (
                    tc.tile_pool(name="wop", bufs=1) as wop,
                    tc.tile_pool(name="ystp", bufs=3) as ystp,
                    tc.tile_pool(name="psy", bufs=1, space="PSUM") as psy,
                ):
                    wo_sb = wop.tile([P, NJ, DM], f32r)
                    nc.sync.dma_start(
                        out=wo_sb[:], in_=wo.rearrange("(j p) d -> p j d", p=P))
                    for dc in range(DMC):
                        for tsl in range(NQS):
                            psyt = psy.tile([P, QS], f32, tag=f"py{tsl}",
                                            name="psyt")
                            for j in range(NJ):
                                nc.tensor.matmul(
                                    psyt[:],
                                    wo_sb[:, j, dc * P:(dc + 1) * P],
                                    oT[:, j, tsl * QS:(tsl + 1) * QS],
                                    start=(j == 0), stop=(j == NJ - 1))
                            yst = ystp.tile([P, QS], f32, tag="yst", name="yst")
                            nc.vector.tensor_copy(out=yst[:], in_=psyt[:])
                            nc.sync.dma_start(
                                out=yT[dc * P:(dc + 1) * P,
                                       tsl * QS:(tsl + 1) * QS],
                                in_=yst[:])
    _split_sync_waits(nc)
    return nc


def kernel(Q, K, V, W_Q, W_K, W_V, W_O):
    from concourse.bass_utils import run_bass_kernel_spmd

    if "nc" not in _CACHE:
        _CACHE["nc"] = _build_nc()
    nc = _CACHE["nc"]

    Q = np.ascontiguousarray(np.asarray(Q, dtype=np.float32))
    K = np.ascontiguousarray(np.asarray(K, dtype=np.float32))
    V = np.ascontiguousarray(np.asarray(V, dtype=np.float32))
    W_Q = np.asarray(W_Q, dtype=np.float32)
    W_K = np.asarray(W_K, dtype=np.float32)
    W_V = np.asarray(W_V, dtype=np.float32)
    W_O = np.asarray(W_O, dtype=np.float32)

    in_maps = []
    for core in range(8):
        b, g = divmod(core, 2)
        sl = slice(g * NL, (g + 1) * NL)
        in_maps.append({
            "xq": np.ascontiguousarray(Q[b]),
            "xk": np.ascontiguousarray(K[b]),
            "xv": np.ascontiguousarray(V[b]),
            "wq": np.ascontiguousarray(W_Q[:, sl]),
            "wk": np.ascontiguousarray(W_K[:, sl]),
            "wv": np.ascontiguousarray(W_V[:, sl]),
            "wo": np.ascontiguousarray(W_O[sl, :]),
            "ident": np.eye(P, dtype=np.float32),
        })

    res = run_bass_kernel_spmd(nc, in_maps, core_ids=list(range(8)))
    _CACHE["last_result"] = res

    out = np.empty((4, N, DM), np.float32)
    for b in range(4):
        out[b] = (res.results[2 * b]["yT"] + res.results[2 * b + 1]["yT"]).T
    return out
